# revision 1
# baseline (speedup 1.0000x reference)
"""Memory-efficient multi-head cross-attention on 8 TRN2 NeuronCores.

Sharding: batch (2) x head-block (4 heads each) across 8 cores, tensor-parallel
qkv projections.  Each core computes attention context for its 4 heads over
all 2048 query rows; one 8-wide AllToAll per 512-row chunk ships the
normalized context slices to the row-owning cores (cross-batch blocks are
masked out with a host-supplied 0/1 group mask so the program stays
SPMD-uniform), after which the full-depth o-projection, residual add and
LayerNorm for each core's own 128 rows per chunk are entirely local.

Schedule: the attention inner loop is EXP-bound on the Scalar engine
(~147us floor for 16.8M exps per core), so everything else (projections,
o-proj, softmax normalization, residual+LN epilogue) is arranged to hide
inside that stream:
 - per k-tile: one row-tiled score matmul pair (two K=64 matmuls
   co-resident in the PE array via tile_position), ONE wide exp [128,1024]
   covering both heads, and a pair of 65-column context matmuls whose
   extra ones-column accumulates the softmax denominator for free.
 - score psum double-buffered so exp(kt) overlaps matmuls(kt+1).
 - V-proj tail + Q'-proj for chunks 1-3 run as PE fillers inside chunk 0's
   attention; o-proj/normalize for chunk j fills chunk j+1; the
   residual+LN epilogue for chunk j (DVE-only; rsqrt via Newton, no ACT
   table switch) fills chunk j+2.
 - the prefix is DMA-bound, so inputs are trimmed (bf16 residual, no
   gamma/beta load when they are trivial) and ordered so the K'/V/Q'
   critical set streams first.

kernel(**inputs) takes the FULL unsharded inputs and returns the FULL output.
"""

import sys
import types
from collections import deque

import ml_dtypes
import numpy as np

# ---------------------------------------------------------------------------
# Environment shims (must run before concourse imports are used)
# ---------------------------------------------------------------------------


def _install_ntff_shim():
    """Provide antenv.axon_hooks (absent in this image) so that
    run_bass_kernel_spmd(trace=True) can capture NTFF profiles via the
    axon ctypes hook. Harmless when tracing is off."""
    if "antenv.axon_hooks" in sys.modules:
        return
    hook = None
    try:
        from trn_agent_boot.trn_boot import _ntff_profile_via_ctypes

        hook = _ntff_profile_via_ctypes("/opt/axon/libaxon_pjrt.so")
    except Exception:
        hook = None
    mod = types.ModuleType("antenv.axon_hooks")
    mod.get_axon_ntff_profile_hook = lambda: hook
    mod.set_axon_ntff_profile_hook = lambda h: None
    sys.modules["antenv.axon_hooks"] = mod


_install_ntff_shim()

import concourse.bass as bass  # noqa: E402
import concourse.mybir as mybir  # noqa: E402
import concourse.tile as tile  # noqa: E402
from concourse.bass_utils import run_bass_kernel_spmd  # noqa: E402
from concourse.vector_clock import ScopedClock  # noqa: E402


def _patched_drain_and_barrier(self, tick_clock, wait_clock):
    """The walrus build in this image rejects a Drain carrying multiple sem
    waits ("Too many sync wait commands").  Emit the kernel-tail waits as
    standalone wait instructions on the sync engine instead, then drain."""
    nc = self.nc
    probe = nc.sync.nop(nofuse=True)
    wait_clock.add_sem_waits(probe.ins, ScopedClock({None: tick_clock.global_clock}))
    waits = list(probe.ins.sync_info.on_wait)
    probe.ins.sync_info.on_wait.clear()
    name2sem = {s.name: s for s in self.sems.allocated().values()}
    for w in waits:
        nc.sync.wait_ge(name2sem[w.ant_name], w.wait_value)
    nc.sync.drain()
    nc.all_engine_barrier()
    popped = nc._tile_sem_poison_stack.pop()
    assert popped is self._sem_poison
    nc.clear_and_free_semaphores(list(self.sems.allocated().values()))
    nc.all_engine_barrier()


tile.TileContext._drain_and_barrier = _patched_drain_and_barrier

# Max sem-waits this walrus build accepts on a single instruction.
_WAIT_LIMIT = 1


def _split_waits(nc, limit=_WAIT_LIMIT):
    """Hoist excess per-instruction sem waits into standalone EventSemaphore
    instructions (same engine, immediately preceding), since this walrus build
    rejects instructions carrying more than one sync wait."""
    n_split = 0
    for f in nc.m.functions:
        for bb in f.blocks:
            insts = bb.instructions
            i = 0
            while i < len(insts):
                inst = insts[i]
                si = getattr(inst, "sync_info", None)
                waits = si.on_wait if si is not None else None
                if waits is not None and len(waits) > limit:
                    excess = list(waits)[limit:]
                    del waits[limit:]
                    for w in excess:
                        ev = mybir.InstEventSemaphore(
                            name=f"I-{nc.next_id()}",
                            engine=inst.engine,
                            ins=[],
                            outs=[],
                        )
                        ev.sync_info = mybir.SyncInfo(on_wait=[w], on_update=[])
                        insts.insert(i, ev)
                        i += 1
                        n_split += 1
                i += 1
    return n_split


# ---------------------------------------------------------------------------
# Problem constants (hardcoded per the harness contract)
# ---------------------------------------------------------------------------
B = 2
SQ = 2048
SKV = 2048
D = 1024
NH = 16
DK = 64

NCORES = 8
GSZ = 4  # cores per batch group
HLOC = 4  # heads per core
DLOC = HLOC * DK  # 256 local context channels
P = 128
QCH = 512  # q chunk (matmul moving free dim)
NQC = SQ // QCH  # 4
NKT = SKV // P  # 16 k tiles
NMT = D // P  # 8 contraction tiles over model dim
QTR = 32  # rows per core per RS quarter

F32 = mybir.dt.float32
BF16 = mybir.dt.bfloat16

LN_EPS = 1e-5

_CACHE = {}
LAST_RESULT = None


def _build(apply_gb):
    """Build the SPMD Bass program (identical on all 8 cores)."""
    nc = bass.Bass("TRN2", target_bir_lowering=False, num_devices=NCORES)

    # ---- kernel I/O (per-core shards supplied by the host) ----
    xqT = nc.dram_tensor("xqT", [NQC * NMT, P, QCH], BF16, kind="ExternalInput")
    xkvT = nc.dram_tensor("xkvT", [NMT, P, SKV], BF16, kind="ExternalInput")
    wqT = nc.dram_tensor("wqT", [P, NMT, DLOC], BF16, kind="ExternalInput")
    wkT = nc.dram_tensor("wkT", [P, NMT, DLOC], BF16, kind="ExternalInput")
    wvT = nc.dram_tensor("wvT", [P, NMT, DLOC], BF16, kind="ExternalInput")
    bqs = nc.dram_tensor("bqs", [P, 2], F32, kind="ExternalInput")
    bks = nc.dram_tensor("bks", [P, 2], F32, kind="ExternalInput")
    bvr = nc.dram_tensor("bvr", [1, DLOC], BF16, kind="ExternalInput")
    woT = nc.dram_tensor("woT", [P, NMT, D], BF16, kind="ExternalInput")
    gmsk = nc.dram_tensor("gmsk", [P, 2], F32, kind="ExternalInput")
    # residual rows (query + b_o) for this core: [jc, 128, D]
    qres = nc.dram_tensor("qres", [P, NQC, D], BF16, kind="ExternalInput")
    if apply_gb:
        gam = nc.dram_tensor("gam", [P, D], F32, kind="ExternalInput")
        bet = nc.dram_tensor("bet", [P, D], F32, kind="ExternalInput")
    out = nc.dram_tensor("out", [NQC, P, D], BF16, kind="ExternalOutput")

    groups = [[0, 1, 2, 3], [4, 5, 6, 7]]
    Exp = mybir.ActivationFunctionType.Exp
    MUL = mybir.AluOpType.mult
    ADD = mybir.AluOpType.add
    SUB = mybir.AluOpType.subtract

    with tile.TileContext(nc) as tc:
        with (
            tc.tile_pool(name="cpool", bufs=1) as cpool,
            tc.tile_pool(name="spool", bufs=2) as spool,
            tc.tile_pool(name="dram", bufs=1, space="DRAM") as dram,
        ):
            # ---- persistent SBUF tensors ----
            wq_sb = cpool.tile([P, NMT, DLOC], BF16)
            wk_sb = cpool.tile([P, NMT, DLOC], BF16)
            wv_sb = cpool.tile([P, NMT, DLOC], BF16)
            bqs_sb = cpool.tile([P, 2], F32)
            bks_sb = cpool.tile([P, 2], F32)
            bvr_sb = cpool.tile([1, DLOC], BF16)
            onesP = cpool.tile([P, P], BF16)
            warm = cpool.tile([1, 2], F32)
            warm2 = cpool.tile([1, 2], BF16)
            qt_sb = cpool.tile([P, 2, SQ], BF16)  # Q'^T  (d on partitions)
            kt_sb = cpool.tile([P, 2, SKV], BF16)  # K'^T
            # V rows (k on partitions); per head 64 cols of V + a ones column
            # at col 64 so the context matmul accumulates the softmax
            # denominator into psum partition 64 for free.
            v_sb = cpool.tile([P, NKT, HLOC, 65], BF16)
            # normalized local context C^T: [d(128) x head-pair x q]
            ct_sb = cpool.tile([P, 2, SQ], BF16)
            wo_sb = cpool.tile([P, NMT, D], BF16)
            gm_sb = cpool.tile([P, 2], F32)
            gmb = cpool.tile([P, 2, D], BF16)
            qres_sb = cpool.tile([P, NQC, D], BF16)
            if apply_gb:
                gam_sb = cpool.tile([P, D], F32)
                bet_sb = cpool.tile([P, D], F32)
            xkv_mt = [cpool.tile([P, SKV], BF16, name=f"xkv_{i}") for i in range(NMT)]
            # per-(chunk, mt) query slices so chunk 0's Q' only waits on its
            # own 1MB of input
            xq_t = [
                [
                    cpool.tile([P, QCH], BF16, name=f"xq_{j}_{i}")
                    for i in range(NMT)
                ]
                for j in range(NQC)
            ]
            # softmax-denominator scratch rows (memset so the merged
            # reciprocal over partitions 0:65 never sees uninitialized data)
            dcp = cpool.tile([P, QCH], F32)
            rdf = cpool.tile([P, QCH], F32)
            rdb = cpool.tile([P, QCH], BF16)

            # preload the exp table while DMAs stream
            nc.vector.memset(warm[:], 0.0)
            nc.scalar.activation(warm2[:], warm[:], Exp)
            # tiny warmup collective: absorbs inter-core launch skew during
            # the DMA-bound prefix instead of at the first real AllToAll
            wsync_in = dram.tile([NCORES, 16], F32, name="wsync_in")
            wsync_out = dram.tile([NCORES, 16], F32, name="wsync_out")
            wsrc = cpool.tile([NCORES, 16], F32)
            nc.vector.memset(wsrc[:], 0.0)
            nc.gpsimd.dma_start(wsync_in, wsrc[:])
            nc.gpsimd.collective_compute(
                "AllToAll",
                mybir.AluOpType.bypass,
                replica_groups=[list(range(NCORES))],
                ins=[wsync_in.opt()],
                outs=[wsync_out.opt()],
            )

            # ---- input DMAs: critical set (K', V, Q' ch0) first ----
            xkv_v = xkvT.ap()
            xq_v = xqT.ap()
            # strict two-queue priority: K'/V/Q'0 critical set first, the
            # rest queues behind it; gpsimd queue stays clear for collectives
            nc.sync.dma_start(wk_sb[:], wkT.ap())
            for mt in range(5):
                nc.sync.dma_start(xkv_mt[mt][:], xkv_v[mt])
            nc.scalar.dma_start(wv_sb[:], wvT.ap())
            nc.scalar.dma_start(wq_sb[:], wqT.ap())
            for mt in range(5, NMT):
                nc.scalar.dma_start(xkv_mt[mt][:], xkv_v[mt])
            nc.scalar.dma_start(bks_sb[:], bks.ap())
            nc.scalar.dma_start(bqs_sb[:], bqs.ap())
            nc.scalar.dma_start(bvr_sb[:], bvr.ap())
            nc.scalar.dma_start(gm_sb[:], gmsk.ap())
            for mt in range(NMT):
                nc.scalar.dma_start(xq_t[0][mt][:], xq_v[mt])
            # non-critical set: streams during chunk 0's attention
            for jc in range(1, NQC):
                for mt in range(NMT):
                    q = nc.sync if jc == 1 else nc.scalar
                    q.dma_start(xq_t[jc][mt][:], xq_v[NMT * jc + mt])
            nc.sync.dma_start(wo_sb[:], woT.ap())
            nc.scalar.dma_start(qres_sb[:], qres.ap())
            if apply_gb:
                nc.scalar.dma_start(gam_sb[:], gam.ap())
                nc.scalar.dma_start(bet_sb[:], bet.ap())
            nc.vector.memset(onesP[:], 1.0)
            nc.vector.memset(dcp[:], 1.0)
            nc.vector.memset(rdf[:], 1.0)
            nc.vector.memset(gmb[:], 1.0)
            for g in range(2):
                nc.vector.tensor_scalar_mul(
                    gmb[:, g, :], gmb[:, g, :], gm_sb[:, g : g + 1]
                )
            for h in range(HLOC):
                nc.vector.memset(v_sb[:, :, h, 64:65], 1.0)

            # ---------------- Phase A: K' proj, V (kt 0-7), Q' chunk 0 ----
            with tc.tile_pool(name="psA", bufs=8, space="PSUM") as psA:
                # K'^T: out[d_tile(128), q(512)]; m-tile outer so matmuls
                # start as soon as the first input tile lands
                pss = [
                    psA.tile([P, QCH], F32, tag="pj", name=f"pk_{i}")
                    for i in range(8)
                ]
                for mt in range(NMT):
                    for dt in range(2):
                        for qc in range(NQC):
                            nc.tensor.matmul(
                                pss[dt * NQC + qc][:],
                                lhsT=wk_sb[:, mt, P * dt : P * dt + P],
                                rhs=xkv_mt[mt][:, QCH * qc : QCH * qc + QCH],
                                start=(mt == 0),
                                stop=(mt == NMT - 1),
                            )
                for dt in range(2):
                    for qc in range(NQC):
                        nc.vector.tensor_scalar(
                            kt_sb[:, dt, QCH * qc : QCH * qc + QCH],
                            pss[dt * NQC + qc][:],
                            1.0,
                            bks_sb[:, dt : dt + 1],
                            MUL,
                            ADD,
                        )

                # V kt 0-7 up front (kt 8-15 become fillers in chunk 0)
                def v_tile_mms(ps, kt, lo, hi):
                    pv = ps[:, 0:DLOC]
                    for mt in range(lo, hi):
                        nc.tensor.matmul(
                            pv,
                            lhsT=xkv_mt[mt][:, P * kt : P * kt + P],
                            rhs=wv_sb[:, mt, :],
                            start=(mt == 0),
                            stop=False,
                        )
                    if hi == NMT:
                        nc.tensor.matmul(
                            pv,
                            lhsT=onesP[0:1, :],
                            rhs=bvr_sb[0:1, :],
                            start=False,
                            stop=True,
                        )

                def v_copyback(ps, kt):
                    nc.vector.tensor_copy(
                        v_sb[:, kt, :, 0:64],
                        ps[:, 0:DLOC].rearrange("p (h d) -> p h d", d=DK),
                    )

                # Q' chunk 0 (reuses freed K' psum banks)
                pq = [
                    psA.tile([P, QCH], F32, tag="pj", name=f"pq_{i}")
                    for i in range(2)
                ]
                for mt in range(NMT):
                    for dt in range(2):
                        nc.tensor.matmul(
                            pq[dt][:],
                            lhsT=wq_sb[:, mt, P * dt : P * dt + P],
                            rhs=xq_t[0][mt][:],
                            start=(mt == 0),
                            stop=(mt == NMT - 1),
                        )
                for dt in range(2):
                    nc.vector.tensor_scalar(
                        qt_sb[:, dt, 0:QCH],
                        pq[dt][:],
                        0.125,
                        bqs_sb[:, dt : dt + 1],
                        MUL,
                        ADD,
                    )

                for kt in range(NKT // 2):
                    ps = psA.tile([P, QCH], F32, tag="pj", name=f"pv_{kt}")
                    v_tile_mms(ps, kt, 0, NMT)
                    v_copyback(ps, kt)


            # ------- Phase B: exp-bound attention with fillers -------
            with (
                tc.tile_pool(name="opool", bufs=1) as opool,
                tc.tile_pool(name="psB", bufs=1, space="PSUM") as psB,
            ):
                fillers = deque()
                ctf_tiles = {}
                a2a_in_tiles = {}
                x_tiles = {}
                po_cache = {}
                aux_toggle = [0]

                def aux_tile(name):
                    # two auxiliary psum banks, round-robin, so back-to-back
                    # o-proj / broadcast matmuls double-buffer
                    aux_toggle[0] ^= 1
                    tag = "aux" if aux_toggle[0] else "vq"
                    return psB.tile([P, QCH], F32, tag=tag, bufs=1, name=name)

                def po_tile(jc, nch):
                    key = (jc, nch)
                    if key not in po_cache:
                        po_cache[key] = aux_tile(f"po_{jc}_{nch}")
                    return po_cache[key]

                # ---- filler generators ----
                def v_steps(kt):
                    ps_box = {}

                    def a():
                        ps_box["t"] = aux_tile(f"pvf_{kt}")
                        v_tile_mms(ps_box["t"], kt, 0, 4)

                    def b():
                        v_tile_mms(ps_box["t"], kt, 4, NMT)

                    def c():
                        v_copyback(ps_box["t"], kt)

                    return [a, b, c]

                def qproj_steps(jc, dt):
                    qsl = slice(QCH * jc, QCH * jc + QCH)
                    ps_box = {}

                    def a(lo, hi):
                        if lo == 0:
                            ps_box["t"] = aux_tile(f"pqf_{jc}_{dt}")
                        for mt in range(lo, hi):
                            nc.tensor.matmul(
                                ps_box["t"][:],
                                lhsT=wq_sb[:, mt, P * dt : P * dt + P],
                                rhs=xq_t[jc][mt][:],
                                start=(mt == 0),
                                stop=(mt == NMT - 1),
                            )

                    def c():
                        nc.vector.tensor_scalar(
                            qt_sb[:, dt, qsl],
                            ps_box["t"][:],
                            0.125,
                            bqs_sb[:, dt : dt + 1],
                            MUL,
                            ADD,
                        )

                    return [lambda: a(0, 4), lambda: a(4, NMT), c]

                def norm_head(jc, hp, cx0, cx1):
                    """Free the ctx psum banks quickly at a head-pair
                    boundary: copy denominator rows + unnormalized context
                    to SBUF (DVE only, runs inline)."""
                    ctu = spool.tile([P, QCH], BF16, tag="ctu", bufs=3)
                    nc.vector.tensor_copy(dcp[0:1, :], cx0[64:65, :])
                    nc.vector.tensor_copy(dcp[64:65, :], cx1[64:65, :])
                    nc.vector.tensor_copy(ctu[0:64, :], cx0[0:64, :])
                    nc.vector.tensor_copy(ctu[64:128, :], cx1[0:64, :])
                    return ctu

                def norm_steps(jc, hp, ctu, tail=False, out_box=None):
                    """Deferred tail of the normalization: one merged
                    reciprocal (both heads' denominator rows live in
                    partitions 0 and 64), PE row-broadcast, multiply."""
                    qsl = slice(QCH * jc, QCH * jc + QCH)
                    box = {}

                    def s1a():
                        nc.vector.reciprocal(rdf[0:65, 0:256], dcp[0:65, 0:256])
                        nc.vector.tensor_copy(rdb[0:65, 0:256], rdf[0:65, 0:256])

                    def s1b():
                        nc.vector.reciprocal(rdf[0:65, 256:], dcp[0:65, 256:])
                        nc.vector.tensor_copy(rdb[0:65, 256:], rdf[0:65, 256:])

                    def s2():
                        if tail:
                            # exp stream is done; borrow a score psum bank so
                            # the held po banks stay untouched
                            bcp = psB.tile(
                                [P, 2, QCH], F32, tag="s", bufs=2,
                                name=f"bcpt_{jc}_{hp}",
                            )[:, 0, :]
                        else:
                            bcp = aux_tile(f"bcp_{jc}_{hp}")
                        if out_box is not None:
                            out_box["bcp"] = bcp
                        nc.tensor.matmul(
                            bcp[0:64, :], lhsT=onesP[0:1, 0:64], rhs=rdb[0:1, :]
                        )
                        nc.tensor.matmul(
                            bcp[64:128, :],
                            lhsT=onesP[64:65, 0:64],
                            rhs=rdb[64:65, :],
                        )
                        box["bcp"] = bcp

                    def s3():
                        nc.vector.tensor_mul(
                            ct_sb[:, hp, qsl], ctu[:], box["bcp"][:]
                        )

                    return [s1a, s1b, s2, s3]

                def exchange_steps(jc, hps=(0, 1), tail=False, skip_stage=False):
                    """Ship chunk jc's normalized context through one 8-wide
                    AllToAll (block j = our ctx for the q-rows owned by rank
                    j's position in its group).  Cross-batch blocks arrive as
                    garbage; the masked combine (host-supplied 0/1 per-group
                    mask) keeps only the four same-group blocks, so the
                    program stays SPMD-uniform.  `hps` selects which
                    head-pair halves ship (the last chunk ships per-half so
                    most of its exchange hides inside the exp stream)."""
                    nh = len(hps)
                    sfx = f"{jc}_{hps[0]}{nh}"
                    a2a_in = dram.tile([NCORES, P, nh, P], BF16, name=f"a2a_in_{sfx}")
                    a2a_out = dram.tile([NCORES, P, nh, P], BF16, name=f"a2a_out_{sfx}")
                    ctf8 = opool.tile([P, NCORES, nh, P], BF16, tag=f"c8_{nh}", bufs=2)
                    ctf = opool.tile([P, GSZ, nh, P], BF16, tag=f"cf_{nh}", bufs=2)
                    for i, t in enumerate(hps):
                        ctf_tiles[(jc, t)] = (ctf, i)

                    def st(lo):
                        for j in range(lo, lo + 4):
                            qo = QCH * jc + P * (j % GSZ)
                            nc.sync.dma_start(
                                a2a_in[j],
                                ct_sb[:, hps[0] : hps[0] + nh, qo : qo + P],
                            )

                    def a2a():
                        nc.gpsimd.collective_compute(
                            "AllToAll",
                            mybir.AluOpType.bypass,
                            replica_groups=[list(range(NCORES))],
                            ins=[a2a_in.opt()],
                            outs=[a2a_out.opt()],
                        )

                    def load():
                        av = a2a_out.rearrange("r p t q -> p r t q")
                        nc.sync.dma_start(ctf8[:, 0:GSZ, :, :], av[:, 0:GSZ])
                        nc.sync.dma_start(ctf8[:, GSZ:, :, :], av[:, GSZ:])

                    fsz = GSZ * nh * P

                    def gmv(g):
                        return gmb[:, g, 0:fsz].rearrange(
                            "p (a b c) -> p a b c", a=GSZ, b=nh
                        )

                    eng = nc.vector if tail else nc.gpsimd

                    def comb1():
                        eng.tensor_tensor(
                            ctf8[:, 0:GSZ, :, :], ctf8[:, 0:GSZ, :, :], gmv(0), MUL
                        )

                    def comb2():
                        eng.tensor_tensor(
                            ctf[:], ctf8[:, GSZ : 2 * GSZ, :, :], gmv(1), MUL
                        )
                        eng.tensor_add(ctf[:], ctf[:], ctf8[:, 0:GSZ, :, :])

                    a2a_in_tiles[(jc, hps[0], nh)] = a2a_in
                    if skip_stage:
                        return [a2a, load, comb1, comb2]
                    return [lambda: st(0), lambda: st(4), a2a, load, comb1, comb2]

                def oproj_t_steps(jc, t):
                    """Half of the full-depth o-projection for our own 128
                    rows of chunk jc: the four peers' head-pair-t context
                    blocks accumulate into the held po psum banks.  One
                    matmul per step so a popped filler never starves the
                    exp stream."""

                    def mm1(nch, r):
                        ctf, li = ctf_tiles[(jc, t)]
                        po = po_tile(jc, nch)
                        nsl = slice(QCH * nch, QCH * nch + QCH)
                        nc.tensor.matmul(
                            po[:],
                            lhsT=ctf[:, r, li, :],
                            rhs=wo_sb[:, 2 * r + t, nsl],
                            start=(t == 0 and r == 0),
                            stop=(t == 1 and r == GSZ - 1),
                        )

                    def grp(nch):
                        for r in range(GSZ):
                            mm1(nch, r)

                    return [lambda: grp(0), lambda: grp(1)]

                def oproj_adds(jc):
                    x_sb = opool.tile([P, D], F32, tag="x", bufs=2)
                    x_tiles[jc] = x_sb

                    def add(nch):
                        po = po_tile(jc, nch)
                        nsl = slice(QCH * nch, QCH * nch + QCH)
                        nc.vector.tensor_add(
                            x_sb[:, nsl], po[:], qres_sb[:, jc, nsl]
                        )

                    return [lambda nch=nch: add(nch) for nch in range(2)]

                def epilogue_steps(jc):
                    """LayerNorm for chunk jc's 128 rows.  DVE-only (rsqrt
                    via reciprocal seed + Newton) so the Scalar engine never
                    reloads activation tables."""
                    x_sb = x_tiles[jc]
                    y_sb = opool.tile([P, D], F32, tag="y", bufs=2)
                    yb_sb = opool.tile([P, D], BF16, tag="yb", bufs=2)
                    stat = spool.tile([P, 2, 6], F32, tag="stat")
                    mv = spool.tile([P, 2], F32, tag="mv")
                    var = spool.tile([P, 1], F32, tag="var")
                    yy = spool.tile([P, 1], F32, tag="yy")
                    tt = spool.tile([P, 1], F32, tag="tt")
                    vh = spool.tile([P, 1], F32, tag="vh")
                    mu = mv[:, 0:1]

                    def e2():
                        # mean/variance via the BN stats unit (512-wide max)
                        nc.vector.bn_stats(stat[:, 0, :], x_sb[:, 0 : D // 2])
                        nc.vector.bn_stats(stat[:, 1, :], x_sb[:, D // 2 :])

                    def e3():
                        nc.vector.bn_aggr(mv[:], stat[:])

                    def e4():
                        nc.vector.tensor_scalar_add(var[:], mv[:, 1:2], LN_EPS)
                        nc.vector.reciprocal(yy[:], var[:])
                        nc.vector.tensor_scalar_mul(vh[:], var[:], -0.5)
                        for _ in range(3):
                            nc.vector.tensor_mul(tt[:], yy[:], yy[:])
                            nc.vector.tensor_scalar(tt[:], tt[:], vh[:], 1.5, MUL, ADD)
                            nc.vector.tensor_mul(yy[:], yy[:], tt[:])

                    def e5():
                        if apply_gb:
                            nc.vector.tensor_scalar(
                                y_sb[:], x_sb[:], mu, yy[:], SUB, MUL
                            )
                            nc.vector.tensor_mul(y_sb[:], y_sb[:], gam_sb[:])
                            nc.vector.tensor_add(yb_sb[:], y_sb[:], bet_sb[:])
                            nc.sync.dma_start(out.ap()[jc], yb_sb[:])
                        else:
                            for h in range(2):
                                csl = slice(D // 2 * h, D // 2 * (h + 1))
                                nc.vector.tensor_scalar(
                                    yb_sb[:, csl], x_sb[:, csl], mu, yy[:], SUB, MUL
                                )
                                nc.sync.dma_start(out.ap()[jc][:, csl], yb_sb[:, csl])

                    return [e2, e3, e4, e5]

                # V k-tiles 8-15 and Q' chunks 1-3 fill chunk 0's attention
                for kt in range(NKT // 2, NKT):
                    fillers.extend(v_steps(kt))
                for dt in range(2):
                    fillers.extend(qproj_steps(1, dt))

                # ---- the exp-bound attention loop ----
                def post_steps(k):
                    return (
                        oproj_t_steps(k, 0)
                        + oproj_t_steps(k, 1)
                        + oproj_adds(k)
                        + epilogue_steps(k)
                    )

                pend = {}
                for jc in range(NQC):
                    if jc >= 1:
                        nst = norm_steps(jc - 1, 1, pend[(jc - 1, 1)])
                        fillers.extend(nst[:2])  # reciprocal halves
                        if jc == NQC - 1:
                            # chunk 1's post-exchange work: 1.5 chunks after
                            # its AllToAll so collective jitter never
                            # head-blocks the PE queue
                            fillers.extend(post_steps(1))
                        fillers.extend(nst[2:])  # bcp broadcast + multiply
                        fillers.extend(exchange_steps(jc - 1))
                        if jc + 1 < NQC:
                            # Q' for chunk jc+1: chunk 0 is PE-bound, later
                            # chunks have spare PE slack
                            for dt in range(2):
                                fillers.extend(qproj_steps(jc + 1, dt))
                    qsl = slice(QCH * jc, QCH * jc + QCH)
                    for hp in range(2):
                        if hp == 1:
                            nst = norm_steps(jc, 0, pend[(jc, 0)])
                            fillers.extend(nst[:2])
                            fillers.extend(nst[2:])
                            if jc == NQC - 2:
                                fillers.extend(post_steps(0))
                            if jc == NQC - 1:
                                fillers.extend(post_steps(jc - 1))
                                fillers.extend(exchange_steps(jc, hps=(0,)))
                        cx0 = psB.tile([P, QCH], F32, tag="ctx0", bufs=1)
                        cx1 = psB.tile([P, QCH], F32, tag="ctx1", bufs=1)
                        h0, h1 = 2 * hp, 2 * hp + 1
                        for kt in range(NKT):
                            ksl = slice(P * kt, P * kt + P)
                            s = psB.tile([P, 2, QCH], F32, tag="s", bufs=2)
                            nc.tensor.matmul(
                                s[:, 0, :],
                                lhsT=kt_sb[0:DK, hp, ksl],
                                rhs=qt_sb[0:DK, hp, qsl],
                            )
                            nc.tensor.matmul(
                                s[:, 1, :],
                                lhsT=kt_sb[DK:P, hp, ksl],
                                rhs=qt_sb[DK:P, hp, qsl],
                                tile_position=(64, 0),
                            )
                            p_kt = spool.tile([P, 2, QCH], BF16, tag="p", bufs=12)
                            nc.scalar.activation(p_kt[:], s[:], Exp)
                            st, sp = kt == 0, kt == NKT - 1
                            nc.tensor.matmul(
                                cx0[0:65, :],
                                lhsT=v_sb[:, kt, h0, :],
                                rhs=p_kt[:, 0, :],
                                start=st,
                                stop=sp,
                            )
                            nc.tensor.matmul(
                                cx1[0:65, :],
                                lhsT=v_sb[:, kt, h1, :],
                                rhs=p_kt[:, 1, :],
                                start=st,
                                stop=sp,
                            )
                            # keep the first/last k-tiles filler-free so the
                            # exp stream never competes at boundaries
                            if 2 <= kt <= 14:
                                n_pop = 2 if len(fillers) > 4 else 1
                                for _ in range(n_pop):
                                    if fillers:
                                        fillers.popleft()()
                        pend[(jc, hp)] = norm_head(jc, hp, cx0, cx1)

                # ---- tail ----
                while fillers:
                    fillers.popleft()()
                jl = NQC - 1
                nbox = {}
                norm_tail = norm_steps(jl, 1, pend[(jl, 1)], tail=True, out_box=nbox)
                # head-pair-0 o-proj first: its exchange completed during the
                # last exp stream, so the PE works while the norm chain runs
                for step in [norm_tail[0]] + oproj_t_steps(jl, 0) + norm_tail[1:3]:
                    step()
                # split the final normalize-multiply into 128-column pieces
                # and launch each piece's AllToAll stage DMAs immediately
                ex = exchange_steps(jl, hps=(1,), tail=True, skip_stage=True)
                a2a_in_t = a2a_in_tiles[(jl, 1, 1)]
                ctu_t = pend[(jl, 1)]
                qoff = QCH * jl
                for qb in range(GSZ):
                    csl = slice(P * qb, P * qb + P)
                    nc.vector.tensor_mul(
                        ct_sb[:, 1, qoff + P * qb : qoff + P * qb + P],
                        ctu_t[:, csl],
                        nbox["bcp"][:, csl],
                    )
                    for j in (qb, qb + GSZ):
                        nc.sync.dma_start(
                            a2a_in_t[j],
                            ct_sb[:, 1:2, qoff + P * qb : qoff + P * qb + P],
                        )
                for step in (
                    ex
                    + oproj_t_steps(jl, 1)
                    + oproj_adds(jl)
                    + epilogue_steps(jl)
                ):
                    step()

    _split_waits(nc)
    return nc


def _prep_inputs(query, key_value, W_qkv, b_qkv, W_o, b_o, ln_gamma, ln_beta,
                 apply_gb):
    bf16 = ml_dtypes.bfloat16
    f32 = np.float32
    query = np.asarray(query, f32)
    key_value = np.asarray(key_value, f32)
    W_qkv = np.asarray(W_qkv, f32)
    b_qkv = np.asarray(b_qkv, f32)
    W_o = np.asarray(W_o, f32)
    b_o = np.asarray(b_o, f32)
    ln_gamma = np.asarray(ln_gamma, f32)
    ln_beta = np.asarray(ln_beta, f32)

    Wq, Wk, Wv = W_qkv[:D], W_qkv[D : 2 * D], W_qkv[2 * D :]
    bq, bk, bv = b_qkv[:D], b_qkv[D : 2 * D], b_qkv[2 * D :]

    woT_full = np.ascontiguousarray(
        W_o.T.reshape(NMT, P, D).transpose(1, 0, 2)
    ).astype(bf16)
    gam = np.ascontiguousarray(np.broadcast_to(ln_gamma, (P, D))).astype(f32)
    bet = np.ascontiguousarray(np.broadcast_to(ln_beta, (P, D))).astype(f32)

    xqT = [
        np.ascontiguousarray(
            query[b].T.reshape(NMT, P, NQC, QCH)
            .transpose(2, 0, 1, 3)
            .reshape(NQC * NMT, P, QCH)
        ).astype(bf16)
        for b in range(B)
    ]
    xkvT = [
        np.ascontiguousarray(key_value[b].T.reshape(NMT, P, SKV)).astype(bf16)
        for b in range(B)
    ]

    in_maps = []
    for c in range(NCORES):
        b = c // GSZ
        hb = c % GSZ
        jb = c % GSZ
        sl = slice(DLOC * hb, DLOC * hb + DLOC)
        # this core owns rows 512*jc + 128*jb .. +128 of each chunk jc
        res_rows = np.stack(
            [
                query[b, QCH * jc + P * jb : QCH * jc + P * jb + P]
                + b_o[None, :]
                for jc in range(NQC)
            ]
        ).transpose(1, 0, 2)
        gm = np.zeros((P, 2), f32)
        gm[:, b] = 1.0
        im = {
            "xqT": xqT[b],
            "xkvT": xkvT[b],
            "wqT": np.ascontiguousarray(
                Wq[sl].T.reshape(NMT, P, DLOC).transpose(1, 0, 2)
            ).astype(bf16),
            "wkT": np.ascontiguousarray(
                Wk[sl].T.reshape(NMT, P, DLOC).transpose(1, 0, 2)
            ).astype(bf16),
            "wvT": np.ascontiguousarray(
                Wv[sl].T.reshape(NMT, P, DLOC).transpose(1, 0, 2)
            ).astype(bf16),
            "bqs": np.ascontiguousarray(
                (bq[sl] * 0.125).reshape(2, P).T
            ).astype(f32),
            "bks": np.ascontiguousarray(bk[sl].reshape(2, P).T).astype(f32),
            "bvr": bv[sl][None, :].astype(bf16),
            "woT": woT_full,
            "gmsk": gm,
            "qres": res_rows.astype(bf16),
        }
        if apply_gb:
            im["gam"] = gam
            im["bet"] = bet
        in_maps.append(im)
    return in_maps


def kernel(query, key_value, W_qkv, b_qkv, W_o, b_o, ln_gamma, ln_beta):
    global LAST_RESULT
    apply_gb = not (
        np.all(np.asarray(ln_gamma) == 1.0) and np.all(np.asarray(ln_beta) == 0.0)
    )
    key = ("nc", apply_gb)
    if key not in _CACHE:
        _CACHE[key] = _build(apply_gb)
    nc = _CACHE[key]
    in_maps = _prep_inputs(
        query, key_value, W_qkv, b_qkv, W_o, b_o, ln_gamma, ln_beta, apply_gb
    )
    res = run_bass_kernel_spmd(nc, in_maps, core_ids=list(range(NCORES)))
    LAST_RESULT = res
    full = np.empty((B, SQ, D), np.float32)
    for c in range(NCORES):
        b = c // GSZ
        jb = c % GSZ
        o = np.asarray(res.results[c]["out"], np.float32)  # [NQC, P, D]
        for jc in range(NQC):
            r0 = QCH * jc + P * jb
            full[b, r0 : r0 + P] = o[jc]
    return full

